# revision 11
# baseline (speedup 1.0000x reference)
# Trainium2 Bass kernel for nn_CustomStyleLoss (segment-mean + MSE reduction).
#
# loss = sum_rows mean_chunks( (mean_chunk(input) - mean_chunk(style))^2 )
# with rows = 16*512 = 8192, each row = 50*50 = 2500 elems = 25 chunks of 100.
#
# Data-parallel over the row axis: core i gets rows [i*1024, (i+1)*1024).
# Raw Bass (no Tile framework) to avoid the ~10us of start/stop barrier
# overhead. Per core: 4 tiles of [128 x 5000] f32 per tensor (2 rows per
# partition -> 20KB DMA lines for near-peak HBM bandwidth). Input tiles
# stream on the SP HWDGE ring, style tiles on the ACT ring (the two rings
# together keep all 16 SDMA engines at full rate). The DVE does everything
# else with one pass over the data: tensor_tensor_scan computes the running
# sum of (input - style); chunk sums are strided differences of the scan
# output at the 100-element boundaries; tensor_tensor_reduce squares them
# (scale folded in: (SCALE*cs)^2 with SCALE = 1/(100*sqrt(25)) sums directly
# to the loss) and accumulates each tile into one column of a [128 x 4]
# partials tile. Host sums the 8 x [128 x 4] partials.

import sys

if "/opt/trn_rl_repo" not in sys.path:
    sys.path.insert(0, "/opt/trn_rl_repo")

import numpy as np

import concourse.bass as bass
from concourse import mybir
from concourse.bass_utils import run_bass_kernel_spmd

N_CORES = 8
N_ROWS = 8192          # 16 * 512
K = 2500               # 50 * 50
CHUNK = 100
P = 128
ROWS_PER_PART = 2
F = K * ROWS_PER_PART               # 5000 elems / partition line
CPL = F // CHUNK                    # 50 chunks per partition line
ROWS_PER_CORE = N_ROWS // N_CORES   # 1024
ROWS_PER_TILE = P * ROWS_PER_PART   # 256
N_TILES = ROWS_PER_CORE // ROWS_PER_TILE  # 4
N_BUFS = 3
SCALE = 1.0 / (CHUNK * np.sqrt(K // CHUNK))
SCALE2 = float(SCALE * SCALE)

_CACHED_NC = None


def _build_nc():
    nc = bass.Bass(
        "TRN2",
        target_bir_lowering=False,
        debug=False,
        num_devices=N_CORES,
    )
    x = nc.dram_tensor(
        "input", [ROWS_PER_CORE, K], mybir.dt.float32, kind="ExternalInput"
    ).ap()
    s = nc.dram_tensor(
        "style", [ROWS_PER_CORE, K], mybir.dt.float32, kind="ExternalInput"
    ).ap()
    o = nc.dram_tensor(
        "out", [P, N_TILES], mybir.dt.float32, kind="ExternalOutput"
    ).ap()

    def src(t_ap, t):
        r0 = t * ROWS_PER_TILE
        return t_ap[r0 : r0 + ROWS_PER_TILE, :].rearrange(
            "(p r) k -> p (r k)", r=ROWS_PER_PART
        )

    from contextlib import ExitStack

    with ExitStack() as ctx:
        xt = ctx.enter_context(
            nc.sbuf_tensor("xt", [P, N_BUFS, F], mybir.dt.float32)
        )
        st = ctx.enter_context(
            nc.sbuf_tensor("st", [P, N_BUFS, F], mybir.dt.float32)
        )
        sc = ctx.enter_context(nc.sbuf_tensor("sc", [P, F], mybir.dt.float32))
        cs = ctx.enter_context(nc.sbuf_tensor("cs", [P, CPL], mybir.dt.float32))
        sq = ctx.enter_context(nc.sbuf_tensor("sq", [P, CPL], mybir.dt.float32))
        partials = ctx.enter_context(
            nc.sbuf_tensor("partials", [P, N_TILES], mybir.dt.float32)
        )
        # One semaphore per DMA so no completion-ordering assumptions are
        # needed between DMAs on the same ring.
        s_in = [
            ctx.enter_context(nc.semaphore(f"s_in{t}")) for t in range(N_TILES)
        ]
        s_st = [
            ctx.enter_context(nc.semaphore(f"s_st{t}")) for t in range(N_TILES)
        ]
        s_cs = ctx.enter_context(nc.semaphore("s_cs"))
        s_out = ctx.enter_context(nc.semaphore("s_out"))
        block = ctx.enter_context(nc.Block())

        @block.sync
        def _(sync):
            # Input tiles on the SP HWDGE ring. Issue the first N_BUFS
            # immediately; tile t >= N_BUFS reuses slot t % N_BUFS, which is
            # free once tile t - N_BUFS finished its compute (s_cs).
            for t in range(N_TILES):
                if t >= N_BUFS:
                    sync.wait_ge(s_cs, t - N_BUFS + 1)
                sync.dma_start(out=xt[:, t % N_BUFS, :], in_=src(x, t)).then_inc(
                    s_in[t], 16
                )
            # Ship the per-core partial sums once all tiles are reduced.
            sync.wait_ge(s_cs, N_TILES)
            sync.dma_start(out=o, in_=partials[:]).then_inc(s_out, 16)
            sync.wait_ge(s_out, 16)

        @block.scalar
        def _(scalar):
            # Style tiles on the ACT HWDGE ring (nothing else runs on ACT).
            for t in range(N_TILES):
                if t >= N_BUFS:
                    scalar.wait_ge(s_cs, t - N_BUFS + 1)
                scalar.dma_start(out=st[:, t % N_BUFS, :], in_=src(s, t)).then_inc(
                    s_st[t], 16
                )

        @block.vector
        def _(vector):
            for t in range(N_TILES):
                vector.wait_ge(s_in[t], 16)
                vector.wait_ge(s_st[t], 16)
                # sc[:, j] = sum_{i<=j} (xt[:, i] - st[:, i])  (fp32 state)
                nc.vector.tensor_tensor_scan(
                    out=sc[:],
                    data0=xt[:, t % N_BUFS, :],
                    data1=st[:, t % N_BUFS, :],
                    initial=0.0,
                    op0=mybir.AluOpType.add,
                    op1=mybir.AluOpType.subtract,
                )
                vector.drain()
                # chunk sums from scan boundaries:
                # cs[c] = S[100c+99] - S[100c-1]
                hi = sc[:, CHUNK - 1 : F : CHUNK]  # [P, CPL]
                nc.vector.tensor_copy(cs[:, 0:1], hi[:, 0:1])
                nc.vector.tensor_sub(
                    cs[:, 1:CPL], hi[:, 1:CPL], hi[:, 0 : CPL - 1]
                )
                vector.drain()
                # partials[:, t] = sum_c cs[:, c]^2  (SCALE^2 applied on host)
                nc.vector.tensor_mul(sq[:], cs[:], cs[:])
                vector.drain()
                nc.vector.tensor_reduce(
                    out=partials[:, t : t + 1],
                    in_=sq[:],
                    axis=mybir.AxisListType.X,
                    op=mybir.AluOpType.add,
                ).then_inc(s_cs, 1)

    return nc


def _get_nc():
    global _CACHED_NC
    if _CACHED_NC is None:
        _CACHED_NC = _build_nc()
    return _CACHED_NC


def run_sharded(input, style, **run_kwargs):
    """Shard, run on 8 cores, return (scalar loss, BassKernelResults)."""
    nc = _get_nc()
    xi = np.ascontiguousarray(np.asarray(input, dtype=np.float32)).reshape(
        N_ROWS, K
    )
    xs = np.ascontiguousarray(np.asarray(style, dtype=np.float32)).reshape(
        N_ROWS, K
    )
    in_maps = [
        {
            "input": xi[i * ROWS_PER_CORE : (i + 1) * ROWS_PER_CORE],
            "style": xs[i * ROWS_PER_CORE : (i + 1) * ROWS_PER_CORE],
        }
        for i in range(N_CORES)
    ]
    res = run_bass_kernel_spmd(nc, in_maps, list(range(N_CORES)), **run_kwargs)
    total = np.float64(0.0)
    for r in res.results:
        total += r["out"].astype(np.float64).sum()
    return np.array(total * SCALE2, dtype=np.float32), res


def kernel(input, style):
    loss, _ = run_sharded(input, style)
    return loss
